# revision 35
# baseline (speedup 1.0000x reference)
"""Trainium2 Bass kernel for nn_CaptionEncoderC (LSTM caption encoder).

Data-parallel over batch: 8 cores x 32 sequences each. Per core:
  phase 1: x = embed[seq] gather; xg = x @ x2h_w + (x2h_b + h2h_b)  (one big GEMM,
           full PE utilization), stored to DRAM as float32r.
  phase 2: 64-step LSTM recurrence. Gates via fp32r matmuls (1 cyc/row) with the
           hidden state kept transposed (hT tiles) as the stationary operand.
           Activations run in-place on PSUM; cell update via fused
           scalar_tensor_tensor ops that also apply the validity mask
           (sequences are ragged; outputs past each length are zero, which
           also lets the recurrence carry masked state safely).
  tail:    final_hidden = tanh(h_last @ aff_w + aff_b), h_last gathered from the
           written output rows at t = len-1.

Self-contained: hardcodes shapes from the problem spec. kernel(**inputs) takes
full unsharded inputs and returns (hidden_states, memory_states, final_hidden,
mask) like the reference.
"""

import numpy as np

import concourse.bass as bass
import concourse.mybir as mybir
import concourse.tile as tile
from concourse import bacc
from concourse import bass_utils
from concourse.masks import make_identity

F32 = mybir.dt.float32
F32R = mybir.dt.float32r
I32 = mybir.dt.int32
AF = mybir.ActivationFunctionType
OP = mybir.AluOpType

OPT = {
    "csz": 512,             # cell chunk size (must equal the 512-wide psum banks)
    "dve_transpose": True,  # hT via DVE 32x32 block transposes instead of PE
    "xgi_bufs": 2,
    "tmp_bufs": 5,
    "ps2_bufs": 8,
    "dve_add_gates": (0, 1),  # gates whose xg-add runs on DVE instead of id-MM
    "split_o": False,  # halving o-sigmoid/h-STT ops LOST 79us in the model - op overheads beat the chain gain
}

B, T, V, E, H = 256, 64, 10000, 512, 1024
N_CORES = 8
BC = B // N_CORES  # 32 sequences per core
G4 = 4 * H  # 4096 gate columns
KT = H // 128  # 8 k-tiles over H
ET = E // 128  # 4 k-tiles over E
MT = (BC * T) // 128  # 16 token tiles of 128


def build(nc, T=T, KT=KT, ET=ET, MT=MT):
    """Emit the full kernel IR into nc (a bacc.Bacc)."""
    G4 = KT * 128 * 4
    H_ = KT * 128
    E_ = ET * 128
    NB = G4 // 512  # psum banks for gates

    # ---------------- DRAM I/O ----------------
    embed_d = nc.dram_tensor("embed", [V, E_], F32, kind="ExternalInput")
    wx_d = nc.dram_tensor("wx", [E_, G4], F32, kind="ExternalInput")
    wh_d = nc.dram_tensor("wh", [H_, G4], F32, kind="ExternalInput")
    bias_d = nc.dram_tensor("bias", [1, G4], F32, kind="ExternalInput")
    aff_w_d = nc.dram_tensor("aff_w", [H_, H_], F32, kind="ExternalInput")
    aff_b_d = nc.dram_tensor("aff_b", [1, H_], F32, kind="ExternalInput")
    idx_d = nc.dram_tensor("idx", [128, MT], I32, kind="ExternalInput")
    lens_d = nc.dram_tensor("lens", [BC, 1], F32, kind="ExternalInput")
    lastidx_d = nc.dram_tensor("lastidx", [BC, 1], I32, kind="ExternalInput")

    out_h_d = nc.dram_tensor("out_h", [BC * T, H_], F32, kind="ExternalOutput")
    out_c_d = nc.dram_tensor("out_c", [BC * T, H_], F32, kind="ExternalOutput")
    out_f_d = nc.dram_tensor("out_final", [BC, H_], F32, kind="ExternalOutput")
    out_m_d = nc.dram_tensor("out_mask", [BC, T], F32, kind="ExternalOutput")

    xg_d = nc.dram_tensor("xg_scratch", [BC * T, G4], F32R)

    with tile.TileContext(nc) as tc:
        with (
            tc.tile_pool(name="const", bufs=1) as const_p,
            tc.tile_pool(name="state", bufs=1) as state_p,
            tc.tile_pool(name="hbuf", bufs=2) as h_p,
            tc.tile_pool(name="hT", bufs=2 * KT + 2) as hT_p,
        ):
            # ---------------- constants ----------------
            ident = const_p.tile([128, 128], F32)
            make_identity(nc, ident[:])
            ident_r = const_p.tile([128, 128], F32R)
            nc.vector.tensor_copy(ident_r, ident)

            idx_sb = const_p.tile([128, MT], I32)
            nc.sync.dma_start(out=idx_sb, in_=idx_d.ap())
            lens_sb = const_p.tile([BC, 1], F32)
            nc.sync.dma_start(out=lens_sb, in_=lens_d.ap())

            iota_i = const_p.tile([BC, T], I32)
            nc.gpsimd.iota(iota_i[:], pattern=[[1, T]], base=0, channel_multiplier=0)
            iota_f = const_p.tile([BC, T], F32)
            nc.vector.tensor_copy(iota_f, iota_i)
            valid = const_p.tile([BC, T], F32)
            nc.vector.tensor_scalar(
                out=valid[:], in0=iota_f[:], scalar1=lens_sb[:, :1], scalar2=None,
                op0=OP.is_lt,
            )
            nc.sync.dma_start(out=out_m_d.ap(), in_=valid)

            # ---------------- phase 1: xg = x @ wx + bias ----------------
            with (
                tc.tile_pool(name="wx", bufs=1) as wx_p,
                tc.tile_pool(name="wtmp", bufs=2) as wtmp_p,
                tc.tile_pool(name="xin", bufs=3) as x_p,
                tc.tile_pool(name="xT", bufs=2 * ET) as xT_p,
                tc.tile_pool(name="xgout", bufs=OPT.get("xgo_bufs", 2)) as xgo_p,
                tc.tile_pool(name="ps1", bufs=6, space="PSUM") as ps1,
            ):
                def gather(m):
                    x_sb = x_p.tile([128, E_], F32, tag="x")
                    nc.gpsimd.indirect_dma_start(
                        out=x_sb[:],
                        out_offset=None,
                        in_=embed_d.ap(),
                        in_offset=bass.IndirectOffsetOnAxis(ap=idx_sb[:, m:m + 1], axis=0),
                    )
                    return x_sb

                # first gathers go ahead of the Wx loads in the SWDGE queue
                x_pre = [gather(m) for m in range(min(2, MT))]

                wx_sb = []
                for e in range(ET):
                    wxe = wx_p.tile([128, G4], F32R, tag=f"wx{e}")
                    if OPT.get("wx_swdge"):
                        nc.gpsimd.dma_start(
                            out=wxe[:], in_=wx_d.ap()[e * 128:(e + 1) * 128, :],
                        )
                    else:
                        wt = wtmp_p.tile([128, G4], F32, tag="wtmp")
                        nc.sync.dma_start(out=wt, in_=wx_d.ap()[e * 128:(e + 1) * 128, :])
                        nc.vector.tensor_copy(wxe[:], wt)
                    wx_sb.append(wxe)
                bias_sb = wx_p.tile([128, G4], F32)
                nc.sync.dma_start(
                    out=bias_sb, in_=bias_d.ap().to_broadcast([128, G4]),
                )

                for m in range(MT):
                    x_sb = x_pre[m] if m < len(x_pre) else gather(m)
                    xTs = []
                    for e in range(ET):
                        tp = ps1.tile([128, 128], F32, tag="ps1")
                        nc.tensor.transpose(tp[:], x_sb[:, e * 128:(e + 1) * 128], ident[:])
                        xT = xT_p.tile([128, 128], F32R, tag="xT")
                        nc.vector.tensor_copy(xT, tp)
                        xTs.append(xT)
                    xg_sb = xgo_p.tile([128, G4], F32R, tag="xgo")
                    for n in range(NB):
                        pb = ps1.tile([128, 512], F32, tag="ps1")
                        for e in range(ET):
                            nc.tensor.matmul(
                                pb[:], xTs[e][:], wx_sb[e][:, n * 512:(n + 1) * 512],
                                start=(e == 0), stop=(e == ET - 1),
                            )
                        nc.vector.scalar_tensor_tensor(
                            out=xg_sb[:, n * 512:(n + 1) * 512],
                            in0=pb[:], scalar=1.0,
                            in1=bias_sb[:, n * 512:(n + 1) * 512],
                            op0=OP.mult, op1=OP.add,
                        )
                    nc.sync.dma_start(
                        out=xg_d.ap()[m * 128:(m + 1) * 128, :], in_=xg_sb,
                    )

            # ---------------- phase 2: recurrence ----------------
            with tc.tile_pool(name="wh", bufs=1) as wh_p:
                # load gate-block-major so the first step's banks unblock after
                # ~1/4 of the 16MB weight load instead of all of it
                wh_sb = {}
                for g in range(4):
                    for k in range(KT):
                        whkg = wh_p.tile([128, H_], F32R, tag=f"wh{k}_{g}")
                        nc.gpsimd.dma_start(
                            out=whkg[:],
                            in_=wh_d.ap()[k * 128:(k + 1) * 128,
                                          g * H_:(g + 1) * H_],
                        )
                        wh_sb[(k, g)] = whkg

                with (
                    tc.tile_pool(name="xgin", bufs=OPT["xgi_bufs"]) as xgi_p,
                    tc.tile_pool(name="tmp", bufs=OPT["tmp_bufs"]) as tmp_p,
                    tc.tile_pool(name="gbar", bufs=1) as gbar_p,
                    tc.tile_pool(name="ps2", bufs=OPT["ps2_bufs"], space="PSUM") as ps2,
                ):
                    c_st = state_p.tile([BC, H_], F32)
                    nc.vector.memset(c_st[:], 0.0)
                    hT = []
                    for k in range(KT):
                        t0 = hT_p.tile([128, BC], F32R, tag="hT")
                        nc.vector.memset(t0[:].bitcast(F32), 0.0)
                        hT.append(t0)

                    for t in range(T):
                        v_t = valid[:, t:t + 1]
                        xg_sb = xgi_p.tile([BC, G4], F32R, tag="xgi")
                        nc.sync.dma_start(
                            out=xg_sb, in_=xg_d.ap()[t * BC:(t + 1) * BC, :],
                        )

                        nhalf = H_ // 512

                        def gate_half(g, half, act, dst, dsl):
                            # one psum bank: act(h @ wh[:, cols] + xg[cols]).
                            # The xg term either rides the PSUM accumulation as a
                            # final identity matmul (zero DVE cost) or, for the
                            # early gates, is a DVE add off the critical tail -
                            # balancing PE vs DVE load.
                            on_dve = g in OPT["dve_add_gates"]
                            pb = ps2.tile([BC, 512], F32, tag="ps2")
                            col0 = g * H_ + half * 512
                            for k in range(KT):
                                nc.tensor.matmul(
                                    pb[:], hT[k][:],
                                    wh_sb[(k, g)][:, half * 512:(half + 1) * 512],
                                    start=(k == 0), stop=(on_dve and k == KT - 1),
                                )
                            if not on_dve:
                                nc.tensor.matmul(
                                    pb[:], ident_r[:BC, :BC],
                                    xg_sb[:, col0:col0 + 512],
                                    start=False, stop=True,
                                )
                            else:
                                nc.vector.tensor_add(
                                    pb[:], pb[:], xg_sb[:, col0:col0 + 512],
                                )
                            if dst is None:
                                if g == 3 and OPT.get("split_o"):
                                    # two half-width sigmoids: the first one
                                    # unblocks the h-chain ~0.4us sooner
                                    nc.scalar.activation(pb[:, :256], pb[:, :256], act)
                                    nc.scalar.activation(pb[:, 256:], pb[:, 256:], act)
                                else:
                                    nc.scalar.activation(pb[:], pb[:], act)
                                return pb
                            nc.scalar.activation(dst[:, dsl], pb[:], act)
                            return pb

                        # i, f, g gates first (the c-chain needs them); o overlaps it
                        pb_i, pb_f, pb_o = {}, {}, {}
                        gbar = gbar_p.tile([BC, H_], F32, tag="gbar")
                        for half in range(nhalf):
                            hsl = slice(half * 512, (half + 1) * 512)
                            pb_i[half] = gate_half(0, half, AF.Sigmoid, None, None)
                            pb_f[half] = gate_half(1, half, AF.Sigmoid, None, None)
                            gate_half(2, half, AF.Tanh, gbar, hsl)  # tanh(g) -> SBUF

                        h_new = h_p.tile([BC, H_], F32, tag="h")
                        csz = OPT["csz"]
                        nch = H_ // csz

                        def cell_chunk(ch):
                            sl = slice(ch * csz, (ch + 1) * csz)
                            t1 = tmp_p.tile([BC, csz], F32, tag="tmp")
                            nc.vector.scalar_tensor_tensor(
                                out=t1[:], in0=pb_f[ch][:], scalar=v_t,
                                in1=c_st[:, sl], op0=OP.mult, op1=OP.mult,
                            )
                            t2 = tmp_p.tile([BC, csz], F32, tag="tmp")
                            nc.vector.scalar_tensor_tensor(
                                out=t2[:], in0=pb_i[ch][:], scalar=v_t,
                                in1=gbar[:, sl], op0=OP.mult, op1=OP.mult,
                            )
                            nc.vector.tensor_add(c_st[:, sl], t1[:], t2[:])
                            th = tmp_p.tile([BC, csz], F32, tag="tmp")
                            nc.scalar.activation(th[:], c_st[:, sl], AF.Tanh)
                            return th

                        def h_chunk(ch, th):
                            if OPT.get("split_o"):
                                for q in range(2):
                                    qsl = slice(ch * csz + q * 256, ch * csz + (q + 1) * 256)
                                    nc.vector.scalar_tensor_tensor(
                                        out=h_new[:, qsl], in0=th[:, q * 256:(q + 1) * 256],
                                        scalar=v_t, in1=pb_o[ch][:, q * 256:(q + 1) * 256],
                                        op0=OP.mult, op1=OP.mult,
                                    )
                            else:
                                sl = slice(ch * csz, (ch + 1) * csz)
                                nc.vector.scalar_tensor_tensor(
                                    out=h_new[:, sl], in0=th[:], scalar=v_t,
                                    in1=pb_o[ch][:], op0=OP.mult, op1=OP.mult,
                                )

                        def transposes(ch):
                            out = []
                            for k in range(ch * csz // 128, (ch + 1) * csz // 128):
                                ht = hT_p.tile([128, BC], F32R, tag="hT")
                                if OPT["dve_transpose"]:
                                    htf = tmp_p.tile([128, BC], F32, tag="htf")
                                    for a in range(4):
                                        nc.vector.transpose(
                                            htf[a * 32:(a + 1) * 32, :],
                                            h_new[:, k * 128 + a * 32:k * 128 + (a + 1) * 32],
                                        )
                                    nc.vector.tensor_copy(ht, htf)
                                else:
                                    tp = ps2.tile([128, BC], F32, tag="ps2")
                                    nc.tensor.transpose(
                                        tp[:], h_new[:, k * 128:(k + 1) * 128],
                                        ident[:BC, :BC],
                                    )
                                    nc.vector.tensor_copy(ht, tp)
                                out.append(ht)
                            return out

                        ths = [cell_chunk(0)]
                        for half in range(nhalf):
                            pb_o[half] = gate_half(3, half, AF.Sigmoid, None, None)
                        for ch in range(1, nch):
                            ths.append(cell_chunk(ch))
                        hT_next = []
                        for ch in range(nch):
                            h_chunk(ch, ths[ch])
                            hT_next.extend(transposes(ch))
                        hT = hT_next
                        # masked outputs (h_new, c_st already masked)
                        nc.sync.dma_start(
                            out=out_h_d.ap()[t * BC:(t + 1) * BC, :], in_=h_new,
                        )
                        nc.sync.dma_start(
                            out=out_c_d.ap()[t * BC:(t + 1) * BC, :], in_=c_st,
                        )

            # ---------------- tail: final affine ----------------
            with (
                tc.tile_pool(name="aff", bufs=1) as aff_p,
                tc.tile_pool(name="ps3", bufs=4, space="PSUM") as ps3,
            ):
                # gather first: it shares the SWDGE FIFO with the aff_w
                # loads and must not queue behind 4MB of weights
                li_sb = aff_p.tile([BC, 1], I32)
                nc.sync.dma_start(out=li_sb, in_=lastidx_d.ap())
                hl = aff_p.tile([BC, H_], F32)
                nc.gpsimd.indirect_dma_start(
                    out=hl[:],
                    out_offset=None,
                    in_=out_h_d.ap(),
                    in_offset=bass.IndirectOffsetOnAxis(ap=li_sb[:, :1], axis=0),
                )
                aff_sb = []
                for k in range(KT):
                    affk = aff_p.tile([128, H_], F32R, tag=f"aff{k}")
                    nc.gpsimd.dma_start(
                        out=affk[:],
                        in_=aff_w_d.ap()[k * 128:(k + 1) * 128, :],
                    )
                    aff_sb.append(affk)
                affb_sb = aff_p.tile([128, H_], F32)
                nc.sync.dma_start(
                    out=affb_sb, in_=aff_b_d.ap().to_broadcast([128, H_]),
                )
                hlr = aff_p.tile([BC, H_], F32R)
                nc.vector.tensor_copy(hlr, hl)
                hlT = []
                for k in range(KT):
                    tp = ps3.tile([128, BC], F32R, tag="ps3")
                    nc.tensor.transpose(
                        tp[:], hlr[:, k * 128:(k + 1) * 128], ident_r[:BC, :BC],
                    )
                    ht = aff_p.tile([128, BC], F32R, tag=f"hlT{k}")
                    nc.vector.tensor_copy(ht, tp)
                    hlT.append(ht)
                fin = aff_p.tile([BC, H_], F32)
                for n in range(H_ // 512):
                    pb = ps3.tile([BC, 512], F32, tag="ps3")
                    for k in range(KT):
                        nc.tensor.matmul(
                            pb[:], hlT[k][:], aff_sb[k][:, n * 512:(n + 1) * 512],
                            start=(k == 0), stop=(k == KT - 1),
                        )
                    nc.vector.scalar_tensor_tensor(
                        out=fin[:, n * 512:(n + 1) * 512], in0=pb[:], scalar=1.0,
                        in1=affb_sb[:BC, n * 512:(n + 1) * 512],
                        op0=OP.mult, op1=OP.add,
                    )
                nc.scalar.activation(fin[:], fin[:], AF.Tanh)
                nc.sync.dma_start(out=out_f_d.ap(), in_=fin)

    return nc


_CACHE = {}
TRACE = False  # test harness can set True to collect an NTFF profile


def _get_compiled():
    if "nc" not in _CACHE:
        nc = bacc.Bacc("TRN2", target_bir_lowering=False, debug=False)
        build(nc)
        nc.compile()
        _CACHE["nc"] = nc
    return _CACHE["nc"]


def kernel(**inputs):
    seq = np.asarray(inputs["seq"]).astype(np.int32)          # [B, T]
    seq_len = np.asarray(inputs["seq_len"]).astype(np.int32)  # [B, 1]
    embed = np.ascontiguousarray(np.asarray(inputs["embed"], dtype=np.float32))
    x2h_w = np.ascontiguousarray(np.asarray(inputs["x2h_w"], dtype=np.float32))
    x2h_b = np.asarray(inputs["x2h_b"], dtype=np.float32)
    h2h_w = np.ascontiguousarray(np.asarray(inputs["h2h_w"], dtype=np.float32))
    h2h_b = np.asarray(inputs["h2h_b"], dtype=np.float32)
    aff_w = np.ascontiguousarray(np.asarray(inputs["aff_w"], dtype=np.float32))
    aff_b = np.asarray(inputs["aff_b"], dtype=np.float32)

    bias = (x2h_b + h2h_b).reshape(1, G4)
    aff_b2 = aff_b.reshape(1, H)

    nc = _get_compiled()

    in_maps = []
    for c in range(N_CORES):
        sl = slice(c * BC, (c + 1) * BC)
        seq_c = seq[sl]          # [32, 64]
        len_c = seq_len[sl]      # [32, 1]
        # token row r = t*BC + b ; idx[p, m] is row m*128+p
        rows = np.arange(BC * T)
        tok = seq_c[rows % BC, rows // BC].astype(np.int32)   # [2048]
        idx = tok.reshape(MT, 128).T.copy()                   # [128, MT]
        lastidx = ((len_c[:, 0] - 1) * BC + np.arange(BC)).astype(np.int32)
        in_maps.append({
            "embed": embed,
            "wx": x2h_w,
            "wh": h2h_w,
            "bias": bias,
            "aff_w": aff_w,
            "aff_b": aff_b2,
            "idx": np.ascontiguousarray(idx),
            "lens": len_c.astype(np.float32),
            "lastidx": lastidx.reshape(BC, 1),
        })

    res = bass_utils.run_bass_kernel_spmd(
        nc, in_maps, core_ids=list(range(N_CORES)), trace=TRACE,
    )
    _CACHE["last_res"] = res

    hidden = np.empty((B, T, H), np.float32)
    memory = np.empty((B, T, H), np.float32)
    final = np.empty((B, H), np.float32)
    mask = np.empty((B, T), np.float32)
    for c in range(N_CORES):
        o = res.results[c]
        sl = slice(c * BC, (c + 1) * BC)
        hidden[sl] = o["out_h"].reshape(T, BC, H).transpose(1, 0, 2)
        memory[sl] = o["out_c"].reshape(T, BC, H).transpose(1, 0, 2)
        final[sl] = o["out_final"]
        mask[sl] = o["out_mask"]
    return hidden, memory, final, mask


# revision 38
# speedup vs baseline: 1.0009x; 1.0009x over previous
"""Trainium2 Bass kernel for nn_CaptionEncoderC (LSTM caption encoder).

Data-parallel over batch: 8 cores x 32 sequences each. Per core:
  phase 1: x = embed[seq] gather; xg = x @ x2h_w + (x2h_b + h2h_b)  (one big GEMM,
           full PE utilization), stored to DRAM as float32r.
  phase 2: 64-step LSTM recurrence. Gates via fp32r matmuls (1 cyc/row) with the
           hidden state kept transposed (hT tiles) as the stationary operand.
           Activations run in-place on PSUM; cell update via fused
           scalar_tensor_tensor ops that also apply the validity mask
           (sequences are ragged; outputs past each length are zero, which
           also lets the recurrence carry masked state safely).
  tail:    final_hidden = tanh(h_last @ aff_w + aff_b), h_last gathered from the
           written output rows at t = len-1.

Self-contained: hardcodes shapes from the problem spec. kernel(**inputs) takes
full unsharded inputs and returns (hidden_states, memory_states, final_hidden,
mask) like the reference.
"""

import numpy as np

import concourse.bass as bass
import concourse.mybir as mybir
import concourse.tile as tile
from concourse import bacc
from concourse import bass_utils
from concourse.masks import make_identity

F32 = mybir.dt.float32
F32R = mybir.dt.float32r
I32 = mybir.dt.int32
AF = mybir.ActivationFunctionType
OP = mybir.AluOpType

OPT = {
    "csz": 512,             # cell chunk size (must equal the 512-wide psum banks)
    "dve_transpose": True,  # hT via DVE 32x32 block transposes instead of PE
    "xgi_bufs": 2,
    "tmp_bufs": 5,
    "ps2_bufs": 8,
    "dve_add_gates": (0, 1),  # gates whose xg-add runs on DVE instead of id-MM
    "ps1_bufs": 8,
    "split_o": False,  # halving o-sigmoid/h-STT ops LOST 79us in the model - op overheads beat the chain gain
}

B, T, V, E, H = 256, 64, 10000, 512, 1024
N_CORES = 8
BC = B // N_CORES  # 32 sequences per core
G4 = 4 * H  # 4096 gate columns
KT = H // 128  # 8 k-tiles over H
ET = E // 128  # 4 k-tiles over E
MT = (BC * T) // 128  # 16 token tiles of 128


def build(nc, T=T, KT=KT, ET=ET, MT=MT):
    """Emit the full kernel IR into nc (a bacc.Bacc)."""
    G4 = KT * 128 * 4
    H_ = KT * 128
    E_ = ET * 128
    NB = G4 // 512  # psum banks for gates

    # ---------------- DRAM I/O ----------------
    embed_d = nc.dram_tensor("embed", [V, E_], F32, kind="ExternalInput")
    wx_d = nc.dram_tensor("wx", [E_, G4], F32, kind="ExternalInput")
    wh_d = nc.dram_tensor("wh", [H_, G4], F32, kind="ExternalInput")
    bias_d = nc.dram_tensor("bias", [1, G4], F32, kind="ExternalInput")
    aff_w_d = nc.dram_tensor("aff_w", [H_, H_], F32, kind="ExternalInput")
    aff_b_d = nc.dram_tensor("aff_b", [1, H_], F32, kind="ExternalInput")
    idx_d = nc.dram_tensor("idx", [128, MT], I32, kind="ExternalInput")
    lens_d = nc.dram_tensor("lens", [BC, 1], F32, kind="ExternalInput")
    lastidx_d = nc.dram_tensor("lastidx", [BC, 1], I32, kind="ExternalInput")

    out_h_d = nc.dram_tensor("out_h", [BC * T, H_], F32, kind="ExternalOutput")
    out_c_d = nc.dram_tensor("out_c", [BC * T, H_], F32, kind="ExternalOutput")
    out_f_d = nc.dram_tensor("out_final", [BC, H_], F32, kind="ExternalOutput")
    out_m_d = nc.dram_tensor("out_mask", [BC, T], F32, kind="ExternalOutput")

    xg_d = nc.dram_tensor("xg_scratch", [BC * T, G4], F32R)

    with tile.TileContext(nc) as tc:
        with (
            tc.tile_pool(name="const", bufs=1) as const_p,
            tc.tile_pool(name="state", bufs=1) as state_p,
            tc.tile_pool(name="hbuf", bufs=OPT.get("h_bufs", 2)) as h_p,
            tc.tile_pool(name="hT", bufs=OPT.get("hT_bufs", 2 * KT + 2)) as hT_p,
        ):
            # ---------------- constants ----------------
            ident = const_p.tile([128, 128], F32)
            make_identity(nc, ident[:])
            ident_r = const_p.tile([128, 128], F32R)
            nc.vector.tensor_copy(ident_r, ident)

            idx_sb = const_p.tile([128, MT], I32)
            nc.sync.dma_start(out=idx_sb, in_=idx_d.ap())
            lens_sb = const_p.tile([BC, 1], F32)
            nc.sync.dma_start(out=lens_sb, in_=lens_d.ap())

            iota_i = const_p.tile([BC, T], I32)
            nc.gpsimd.iota(iota_i[:], pattern=[[1, T]], base=0, channel_multiplier=0)
            iota_f = const_p.tile([BC, T], F32)
            nc.vector.tensor_copy(iota_f, iota_i)
            valid = const_p.tile([BC, T], F32)
            nc.vector.tensor_scalar(
                out=valid[:], in0=iota_f[:], scalar1=lens_sb[:, :1], scalar2=None,
                op0=OP.is_lt,
            )
            nc.sync.dma_start(out=out_m_d.ap(), in_=valid)

            # ---------------- phase 1: xg = x @ wx + bias ----------------
            with (
                tc.tile_pool(name="wx", bufs=1) as wx_p,
                tc.tile_pool(name="wtmp", bufs=2) as wtmp_p,
                tc.tile_pool(name="xin", bufs=3) as x_p,
                tc.tile_pool(name="xT", bufs=OPT.get("xT_bufs", 2 * ET)) as xT_p,
                tc.tile_pool(name="xgout", bufs=OPT.get("xgo_bufs", 2)) as xgo_p,
                tc.tile_pool(name="ps1", bufs=OPT.get("ps1_bufs", 6), space="PSUM") as ps1,
            ):
                def gather(m):
                    x_sb = x_p.tile([128, E_], F32, tag="x")
                    nc.gpsimd.indirect_dma_start(
                        out=x_sb[:],
                        out_offset=None,
                        in_=embed_d.ap(),
                        in_offset=bass.IndirectOffsetOnAxis(ap=idx_sb[:, m:m + 1], axis=0),
                    )
                    return x_sb

                # first gathers go ahead of the Wx loads in the SWDGE queue
                x_pre = [gather(m) for m in range(min(2, MT))]

                wx_sb = []
                for e in range(ET):
                    wxe = wx_p.tile([128, G4], F32R, tag=f"wx{e}")
                    if OPT.get("wx_swdge"):
                        nc.gpsimd.dma_start(
                            out=wxe[:], in_=wx_d.ap()[e * 128:(e + 1) * 128, :],
                        )
                    else:
                        wt = wtmp_p.tile([128, G4], F32, tag="wtmp")
                        nc.sync.dma_start(out=wt, in_=wx_d.ap()[e * 128:(e + 1) * 128, :])
                        nc.vector.tensor_copy(wxe[:], wt)
                    wx_sb.append(wxe)
                bias_sb = wx_p.tile([128, G4], F32)
                nc.sync.dma_start(
                    out=bias_sb, in_=bias_d.ap().to_broadcast([128, G4]),
                )

                for m in range(MT):
                    x_sb = x_pre[m] if m < len(x_pre) else gather(m)
                    xTs = []
                    for e in range(ET):
                        tp = ps1.tile([128, 128], F32, tag="ps1")
                        nc.tensor.transpose(tp[:], x_sb[:, e * 128:(e + 1) * 128], ident[:])
                        xT = xT_p.tile([128, 128], F32R, tag="xT")
                        nc.vector.tensor_copy(xT, tp)
                        xTs.append(xT)
                    xg_sb = xgo_p.tile([128, G4], F32R, tag="xgo")
                    for n in range(NB):
                        pb = ps1.tile([128, 512], F32, tag="ps1")
                        for e in range(ET):
                            nc.tensor.matmul(
                                pb[:], xTs[e][:], wx_sb[e][:, n * 512:(n + 1) * 512],
                                start=(e == 0), stop=(e == ET - 1),
                            )
                        nc.vector.scalar_tensor_tensor(
                            out=xg_sb[:, n * 512:(n + 1) * 512],
                            in0=pb[:], scalar=1.0,
                            in1=bias_sb[:, n * 512:(n + 1) * 512],
                            op0=OP.mult, op1=OP.add,
                        )
                    nc.sync.dma_start(
                        out=xg_d.ap()[m * 128:(m + 1) * 128, :], in_=xg_sb,
                    )

            # ---------------- phase 2: recurrence ----------------
            with tc.tile_pool(name="wh", bufs=1) as wh_p:
                # load gate-block-major so the first step's banks unblock after
                # ~1/4 of the 16MB weight load instead of all of it
                wh_sb = {}
                for g in range(4):
                    for k in range(KT):
                        whkg = wh_p.tile([128, H_], F32R, tag=f"wh{k}_{g}")
                        nc.gpsimd.dma_start(
                            out=whkg[:],
                            in_=wh_d.ap()[k * 128:(k + 1) * 128,
                                          g * H_:(g + 1) * H_],
                        )
                        wh_sb[(k, g)] = whkg

                with (
                    tc.tile_pool(name="xgin", bufs=OPT["xgi_bufs"]) as xgi_p,
                    tc.tile_pool(name="tmp", bufs=OPT["tmp_bufs"]) as tmp_p,
                    tc.tile_pool(name="gbar", bufs=1) as gbar_p,
                    tc.tile_pool(name="ps2", bufs=OPT["ps2_bufs"], space="PSUM") as ps2,
                ):
                    c_st = state_p.tile([BC, H_], F32)
                    nc.vector.memset(c_st[:], 0.0)
                    hT = []
                    for k in range(KT):
                        t0 = hT_p.tile([128, BC], F32R, tag="hT")
                        nc.vector.memset(t0[:].bitcast(F32), 0.0)
                        hT.append(t0)

                    for t in range(T):
                        v_t = valid[:, t:t + 1]
                        xg_sb = xgi_p.tile([BC, G4], F32R, tag="xgi")
                        nc.sync.dma_start(
                            out=xg_sb, in_=xg_d.ap()[t * BC:(t + 1) * BC, :],
                        )

                        nhalf = H_ // 512

                        def gate_half(g, half, act, dst, dsl):
                            # one psum bank: act(h @ wh[:, cols] + xg[cols]).
                            # The xg term either rides the PSUM accumulation as a
                            # final identity matmul (zero DVE cost) or, for the
                            # early gates, is a DVE add off the critical tail -
                            # balancing PE vs DVE load.
                            on_dve = g in OPT["dve_add_gates"]
                            pb = ps2.tile([BC, 512], F32, tag="ps2")
                            col0 = g * H_ + half * 512
                            for k in range(KT):
                                nc.tensor.matmul(
                                    pb[:], hT[k][:],
                                    wh_sb[(k, g)][:, half * 512:(half + 1) * 512],
                                    start=(k == 0), stop=(on_dve and k == KT - 1),
                                )
                            if not on_dve:
                                nc.tensor.matmul(
                                    pb[:], ident_r[:BC, :BC],
                                    xg_sb[:, col0:col0 + 512],
                                    start=False, stop=True,
                                )
                            else:
                                nc.vector.tensor_add(
                                    pb[:], pb[:], xg_sb[:, col0:col0 + 512],
                                )
                            if dst is None:
                                if g == 3 and OPT.get("split_o"):
                                    # two half-width sigmoids: the first one
                                    # unblocks the h-chain ~0.4us sooner
                                    nc.scalar.activation(pb[:, :256], pb[:, :256], act)
                                    nc.scalar.activation(pb[:, 256:], pb[:, 256:], act)
                                else:
                                    nc.scalar.activation(pb[:], pb[:], act)
                                return pb
                            nc.scalar.activation(dst[:, dsl], pb[:], act)
                            return pb

                        # i, f, g gates first (the c-chain needs them); o overlaps it
                        pb_i, pb_f, pb_o = {}, {}, {}
                        gbar = gbar_p.tile([BC, H_], F32, tag="gbar")
                        for half in range(nhalf):
                            hsl = slice(half * 512, (half + 1) * 512)
                            pb_i[half] = gate_half(0, half, AF.Sigmoid, None, None)
                            pb_f[half] = gate_half(1, half, AF.Sigmoid, None, None)
                            gate_half(2, half, AF.Tanh, gbar, hsl)  # tanh(g) -> SBUF

                        h_new = h_p.tile([BC, H_], F32, tag="h")
                        csz = OPT["csz"]
                        nch = H_ // csz

                        def cell_chunk(ch):
                            sl = slice(ch * csz, (ch + 1) * csz)
                            t1 = tmp_p.tile([BC, csz], F32, tag="tmp")
                            nc.vector.scalar_tensor_tensor(
                                out=t1[:], in0=pb_f[ch][:], scalar=v_t,
                                in1=c_st[:, sl], op0=OP.mult, op1=OP.mult,
                            )
                            t2 = tmp_p.tile([BC, csz], F32, tag="tmp")
                            nc.vector.scalar_tensor_tensor(
                                out=t2[:], in0=pb_i[ch][:], scalar=v_t,
                                in1=gbar[:, sl], op0=OP.mult, op1=OP.mult,
                            )
                            nc.vector.tensor_add(c_st[:, sl], t1[:], t2[:])
                            th = tmp_p.tile([BC, csz], F32, tag="tmp")
                            nc.scalar.activation(th[:], c_st[:, sl], AF.Tanh)
                            return th

                        def h_chunk(ch, th):
                            if OPT.get("split_o"):
                                for q in range(2):
                                    qsl = slice(ch * csz + q * 256, ch * csz + (q + 1) * 256)
                                    nc.vector.scalar_tensor_tensor(
                                        out=h_new[:, qsl], in0=th[:, q * 256:(q + 1) * 256],
                                        scalar=v_t, in1=pb_o[ch][:, q * 256:(q + 1) * 256],
                                        op0=OP.mult, op1=OP.mult,
                                    )
                            else:
                                sl = slice(ch * csz, (ch + 1) * csz)
                                nc.vector.scalar_tensor_tensor(
                                    out=h_new[:, sl], in0=th[:], scalar=v_t,
                                    in1=pb_o[ch][:], op0=OP.mult, op1=OP.mult,
                                )

                        def transposes(ch):
                            out = []
                            for k in range(ch * csz // 128, (ch + 1) * csz // 128):
                                ht = hT_p.tile([128, BC], F32R, tag="hT")
                                if OPT["dve_transpose"]:
                                    htf = tmp_p.tile([128, BC], F32, tag="htf")
                                    for a in range(4):
                                        nc.vector.transpose(
                                            htf[a * 32:(a + 1) * 32, :],
                                            h_new[:, k * 128 + a * 32:k * 128 + (a + 1) * 32],
                                        )
                                    nc.vector.tensor_copy(ht, htf)
                                else:
                                    tp = ps2.tile([128, BC], F32, tag="ps2")
                                    nc.tensor.transpose(
                                        tp[:], h_new[:, k * 128:(k + 1) * 128],
                                        ident[:BC, :BC],
                                    )
                                    nc.vector.tensor_copy(ht, tp)
                                out.append(ht)
                            return out

                        ths = [cell_chunk(0)]
                        for half in range(nhalf):
                            pb_o[half] = gate_half(3, half, AF.Sigmoid, None, None)
                        for ch in range(1, nch):
                            ths.append(cell_chunk(ch))
                        hT_next = []
                        for ch in range(nch):
                            h_chunk(ch, ths[ch])
                            hT_next.extend(transposes(ch))
                        hT = hT_next
                        # masked outputs (h_new, c_st already masked)
                        nc.sync.dma_start(
                            out=out_h_d.ap()[t * BC:(t + 1) * BC, :], in_=h_new,
                        )
                        nc.sync.dma_start(
                            out=out_c_d.ap()[t * BC:(t + 1) * BC, :], in_=c_st,
                        )

            # ---------------- tail: final affine ----------------
            with (
                tc.tile_pool(name="aff", bufs=1) as aff_p,
                tc.tile_pool(name="ps3", bufs=4, space="PSUM") as ps3,
            ):
                # gather first: it shares the SWDGE FIFO with the aff_w
                # loads and must not queue behind 4MB of weights
                li_sb = aff_p.tile([BC, 1], I32)
                nc.sync.dma_start(out=li_sb, in_=lastidx_d.ap())
                hl = aff_p.tile([BC, H_], F32)
                nc.gpsimd.indirect_dma_start(
                    out=hl[:],
                    out_offset=None,
                    in_=out_h_d.ap(),
                    in_offset=bass.IndirectOffsetOnAxis(ap=li_sb[:, :1], axis=0),
                )
                aff_sb = []
                for k in range(KT):
                    affk = aff_p.tile([128, H_], F32R, tag=f"aff{k}")
                    nc.gpsimd.dma_start(
                        out=affk[:],
                        in_=aff_w_d.ap()[k * 128:(k + 1) * 128, :],
                    )
                    aff_sb.append(affk)
                affb_sb = aff_p.tile([128, H_], F32)
                nc.sync.dma_start(
                    out=affb_sb, in_=aff_b_d.ap().to_broadcast([128, H_]),
                )
                hlr = aff_p.tile([BC, H_], F32R)
                nc.vector.tensor_copy(hlr, hl)
                hlT = []
                for k in range(KT):
                    tp = ps3.tile([128, BC], F32R, tag="ps3")
                    nc.tensor.transpose(
                        tp[:], hlr[:, k * 128:(k + 1) * 128], ident_r[:BC, :BC],
                    )
                    ht = aff_p.tile([128, BC], F32R, tag=f"hlT{k}")
                    nc.vector.tensor_copy(ht, tp)
                    hlT.append(ht)
                fin = aff_p.tile([BC, H_], F32)
                for n in range(H_ // 512):
                    pb = ps3.tile([BC, 512], F32, tag="ps3")
                    for k in range(KT):
                        nc.tensor.matmul(
                            pb[:], hlT[k][:], aff_sb[k][:, n * 512:(n + 1) * 512],
                            start=(k == 0), stop=(k == KT - 1),
                        )
                    nc.vector.scalar_tensor_tensor(
                        out=fin[:, n * 512:(n + 1) * 512], in0=pb[:], scalar=1.0,
                        in1=affb_sb[:BC, n * 512:(n + 1) * 512],
                        op0=OP.mult, op1=OP.add,
                    )
                nc.scalar.activation(fin[:], fin[:], AF.Tanh)
                nc.sync.dma_start(out=out_f_d.ap(), in_=fin)

    return nc


_CACHE = {}
TRACE = False  # test harness can set True to collect an NTFF profile


def _get_compiled():
    if "nc" not in _CACHE:
        nc = bacc.Bacc("TRN2", target_bir_lowering=False, debug=False)
        build(nc)
        nc.compile()
        _CACHE["nc"] = nc
    return _CACHE["nc"]


def kernel(**inputs):
    seq = np.asarray(inputs["seq"]).astype(np.int32)          # [B, T]
    seq_len = np.asarray(inputs["seq_len"]).astype(np.int32)  # [B, 1]
    embed = np.ascontiguousarray(np.asarray(inputs["embed"], dtype=np.float32))
    x2h_w = np.ascontiguousarray(np.asarray(inputs["x2h_w"], dtype=np.float32))
    x2h_b = np.asarray(inputs["x2h_b"], dtype=np.float32)
    h2h_w = np.ascontiguousarray(np.asarray(inputs["h2h_w"], dtype=np.float32))
    h2h_b = np.asarray(inputs["h2h_b"], dtype=np.float32)
    aff_w = np.ascontiguousarray(np.asarray(inputs["aff_w"], dtype=np.float32))
    aff_b = np.asarray(inputs["aff_b"], dtype=np.float32)

    bias = (x2h_b + h2h_b).reshape(1, G4)
    aff_b2 = aff_b.reshape(1, H)

    nc = _get_compiled()

    in_maps = []
    for c in range(N_CORES):
        sl = slice(c * BC, (c + 1) * BC)
        seq_c = seq[sl]          # [32, 64]
        len_c = seq_len[sl]      # [32, 1]
        # token row r = t*BC + b ; idx[p, m] is row m*128+p
        rows = np.arange(BC * T)
        tok = seq_c[rows % BC, rows // BC].astype(np.int32)   # [2048]
        idx = tok.reshape(MT, 128).T.copy()                   # [128, MT]
        lastidx = ((len_c[:, 0] - 1) * BC + np.arange(BC)).astype(np.int32)
        in_maps.append({
            "embed": embed,
            "wx": x2h_w,
            "wh": h2h_w,
            "bias": bias,
            "aff_w": aff_w,
            "aff_b": aff_b2,
            "idx": np.ascontiguousarray(idx),
            "lens": len_c.astype(np.float32),
            "lastidx": lastidx.reshape(BC, 1),
        })

    res = bass_utils.run_bass_kernel_spmd(
        nc, in_maps, core_ids=list(range(N_CORES)), trace=TRACE,
    )
    _CACHE["last_res"] = res

    hidden = np.empty((B, T, H), np.float32)
    memory = np.empty((B, T, H), np.float32)
    final = np.empty((B, H), np.float32)
    mask = np.empty((B, T), np.float32)
    for c in range(N_CORES):
        o = res.results[c]
        sl = slice(c * BC, (c + 1) * BC)
        hidden[sl] = o["out_h"].reshape(T, BC, H).transpose(1, 0, 2)
        memory[sl] = o["out_c"].reshape(T, BC, H).transpose(1, 0, 2)
        final[sl] = o["out_final"]
        mask[sl] = o["out_mask"]
    return hidden, memory, final, mask
